# revision 23
# baseline (speedup 1.0000x reference)
"""Capsule routing layer (2 routing iterations) on 8 Trainium2 NeuronCores.

Reference computation:
    priors[b,o,i,h] = sum_d x[b,i,d] * W[o,d,h]          (never materialized)
    iter0: probs = softmax(0) = 1/O
           v0[b,o,h]  = (1/O) * sum_i priors
           out0       = squash(v0)
    logits[b,o,i]     = sum_h priors * out0
    iter1: probs      = softmax(logits, axis=o)
           v1[b,o,h]  = sum_i priors * probs
           return squash(v1)

Algebraic reduction used by this kernel (priors factors out of every use):
    xs[b,d]   = sum_i x[b,i,d]
    v0[b,o,h] = (1/O) sum_d xs[b,d] W[o,d,h]
    g0[b,o]   = sqrt(sn0)/(1+sn0),  sn0 = sum_h v0^2      (squash scale)
    w2[b,o,d] = sum_h W[o,d,h] (g0 * v0[b,o,h])           (g0 folded into v0)
    logits[b,o,i] = sum_d x[b,i,d] w2[b,o,d]
    p         = softmax_o(logits)
    xp[b,o,d] = sum_i p[b,o,i] x[b,i,d]
    v1[b,o,h] = sum_d xp[b,o,d] W[o,d,h]
    out       = squash(v1)

Sharding: data-parallel over batch B=64 across 8 cores, weights replicated.

HBM inputs per core are only xtb [d,b,i] (x transposed, fp16) and
wfb [d,o,h] (weights, fp16).  The other two operand layouts are derived
on-device with the XBAR DMA transpose (no extra HBM traffic):
    xnb  [i, (b,it), d]          moving-side x for the xp matmuls
    wtbP [(o%2,h), o//2, d]      o-pair-packed W^T for the w2 matmuls
v0 and w2 run as o-pair matmuls (128-wide stationaries; for w2 the v0
operand is zero-masked per o-parity) to halve stationary loads.  The exp
and both squash squarings run on the scalar engine; dummy activations
prefetch the Sqrt/Exp table switches off the critical path.  All PSUM
accumulation and the softmax/squash chains stay fp32.
"""

import sys
from contextlib import ExitStack

for _p in ("/opt/trn_rl_repo", "/root/.axon_site/_ro/trn_rl_repo"):
    if _p not in sys.path:
        sys.path.append(_p)

import ml_dtypes
import numpy as np

import concourse.bacc as bacc
import concourse.tile as tile
from concourse import mybir
from concourse import bass_utils
from concourse.masks import make_identity
from concourse.bass import broadcast_tensor_aps

F32 = mybir.dt.float32
F16 = mybir.dt.float16
AF = mybir.ActivationFunctionType
NPF16 = np.float16

# Problem shape (hardcoded per spec)
B, I, DIN = 64, 512, 128
O, H = 32, 64
NCORES = 8
BL = B // NCORES          # 8 local batches per core
P = 128                   # SBUF partitions
ITI = I // P              # 4 i-tiles of 128
BO = BL * O               # 256 (b,o) columns, col = b*O + o
NJ = O // 2               # 16 o-pairs


def capsule_tile_kernel(tc, out_d, xtb_d, xnb_d, wfb_d, wtbP_d):
    with ExitStack() as ctx:
        _capsule_tile_kernel(ctx, tc, out_d, xtb_d, xnb_d, wfb_d, wtbP_d)


def _capsule_tile_kernel(ctx, tc, out_d, xtb_d, xnb_d, wfb_d, wtbP_d):
    nc = tc.nc

    consts = ctx.enter_context(tc.tile_pool(name="consts", bufs=1))
    data = ctx.enter_context(tc.tile_pool(name="data", bufs=1))
    small = ctx.enter_context(tc.tile_pool(name="small", bufs=1))
    pp = ctx.enter_context(tc.tile_pool(name="pp", bufs=1, space="PSUM"))
    plp = ctx.enter_context(tc.tile_pool(name="plp", bufs=3, space="PSUM"))
    efp = ctx.enter_context(tc.tile_pool(name="efp", bufs=3))

    # ---- constants (host-free, run during the DMA window) ----
    ident = consts.tile([H, H], F16)
    make_identity(nc, ident)
    ones64 = consts.tile([H, H], F16)
    nc.gpsimd.memset(ones64, 1.0)
    # block-diagonal ones: partition-block reduce for the paired v0 layout
    bones = consts.tile([P, P], F16)
    nc.gpsimd.memset(bones, 1.0)
    nc.gpsimd.memset(bones[0:H, H:P], 0.0)
    nc.gpsimd.memset(bones[H:P, 0:H], 0.0)
    # zero-masked v0 staging (slot 0: even o, partitions 0-63; slot 1: odd)
    v0sz = data.tile([P, 2, P], F16)
    nc.gpsimd.memset(v0sz, 0.0)
    # ACT table prewarm scratch
    dumin = consts.tile([1, 2], F32)
    nc.gpsimd.memset(dumin, 1.0)
    dumout = consts.tile([1, 2], F32)

    # ---- loads: only xtb + wfb come from HBM (contiguous halves so the
    # descriptors spray wide); xnb and wtbP are XBAR transposes in SBUF,
    # halving chip-level HBM traffic (weights/x are pulled by all 8 cores) ----
    xtb = data.tile([P, BL, I], F16)
    nc.sync.dma_start(out=xtb[:, 0:4], in_=xtb_d[:, 0:4])
    nc.sync.dma_start(out=xtb[:, 4:8], in_=xtb_d[:, 4:8])
    wfb = consts.tile([P, O, H], F16)
    nc.scalar.dma_start(out=wfb, in_=wfb_d)
    # wtbP[(o%2)*64+h, o//2, d] = W[o,d,h]
    wtbP = consts.tile([P, NJ, DIN], F16)
    nc.scalar.dma_start(out=wtbP, in_=wtbP_d)
    # xnb[i, b*4+it, d] = x[b, it*128+i, d]
    xnb = data.tile([P, BL * ITI, DIN], F16)
    nc.sync.dma_start(out=xnb, in_=xnb_d)

    # ---- xs[d, b] = sum_i x / 32: b0-3 on DVE, b4-7 on the scalar engine
    # (Identity activation with per-partition accumulator) ----
    xsf = small.tile([P, BL], F32)
    xsb = small.tile([P, BL], F16)
    xscr = data.tile([P, I], F16)
    xtv = xtb.rearrange("d b (t i) -> d b t i", i=P)
    nc.vector.reduce_sum(xsf[:, 0:4], xtb[:, 0:4], axis=mybir.AxisListType.X)
    for b in range(4, BL):
        nc.scalar.activation(xscr, xtb[:, b], AF.Identity,
                             accum_out=xsf[:, b:b + 1])
    nc.vector.tensor_scalar_mul(xsb, xsf, 1.0 / O)

    # ---- v0 o-pairs: [128=(o%2,h), 128=(j,b)] = wfb-pair^T @ xs ----
    wfb2 = wfb.rearrange("d o h -> d (o h)")
    psv0 = pp.tile([P, P], F32, tag="v0")
    for j in range(NJ):
        nc.tensor.matmul(psv0[:, j * BL:(j + 1) * BL],
                         wfb2[:, j * P:(j + 1) * P], xsb,
                         start=True, stop=True)

    # ---- squash scale g0 (pair layout; Square/Sqrt on scalar engine) ----
    nc.scalar.activation(dumout, dumin, AF.Sqrt)          # prefetch sqrt table
    sq0 = data.tile([P, P], F16)
    nc.scalar.activation(sq0, psv0, AF.Square)
    psg = pp.tile([P, P], F32, tag="g")
    nc.tensor.matmul(psg, bones, sq0, start=True, stop=True)
    rt0 = data.tile([P, P], F32)
    nc.scalar.activation(rt0, psg, AF.Sqrt)
    nc.scalar.activation(dumout, dumin, AF.Exp)           # prefetch exp table
    dn0 = data.tile([P, P], F32)
    nc.vector.tensor_scalar_add(dn0, psg, 1.0)
    rdn0 = data.tile([P, P], F32)
    nc.vector.reciprocal(rdn0, dn0)
    g0p = data.tile([P, P], F32)
    nc.vector.tensor_mul(g0p, rt0, rdn0)

    # masked scaled copies: v0sz[:, par] holds g0*v0 for parity par, else 0
    nc.vector.tensor_mul(v0sz[0:H, 0, :], psv0[0:H, :], g0p[0:H, :])
    nc.vector.tensor_mul(v0sz[H:P, 1, :], psv0[H:P, :], g0p[H:P, :])

    # ---- w2[d,(b,o)] = wtbP_j^T @ masked v0 (contract (o%2,h)) ----
    psw2 = pp.tile([P, BO], F32, tag="w2", bufs=2)
    w2v = psw2.rearrange("d (b o) -> d o b", o=O)
    for j in range(NJ):
        sl = slice(j * BL, (j + 1) * BL)
        nc.tensor.matmul(w2v[:, 2 * j, :], wtbP[:, j, :], v0sz[:, 0, sl],
                         start=True, stop=True)
        nc.tensor.matmul(w2v[:, 2 * j + 1, :], wtbP[:, j, :], v0sz[:, 1, sl],
                         start=True, stop=True)
    w2s = data.tile([P, BO], F16)
    nc.vector.tensor_copy(w2s[:, 0:P], psw2[:, 0:P])
    nc.vector.tensor_copy(w2s[:, P:BO], psw2[:, P:BO])

    # ---- logits for all b (PE), then softmax (ACT exp + DVE + gpsimd) ----
    psls = []
    for b in range(BL):
        psl = plp.tile([P, ITI, O], F32, tag="psl")
        psls.append(psl)
        for it in range(ITI):
            nc.tensor.matmul(psl[:, it, :], xtv[:, b, it, :],
                             w2s[:, b * O:(b + 1) * O], start=True, stop=True)
    esum = small.tile([P, BL, ITI], F32)
    rs = small.tile([P, BL, ITI, 1], F32)
    probs = data.tile([P, BL, ITI, O], F16)
    for b in range(BL):
        ef = efp.tile([P, ITI, O], F32, tag="ef")
        nc.scalar.activation(ef, psls[b], AF.Exp)
        nc.vector.reduce_sum(esum[:, b], ef, axis=mybir.AxisListType.X)
        nc.vector.reciprocal(rs[:, b, :, 0], esum[:, b])
        ef_ap, rs_ap = broadcast_tensor_aps(ef[:], rs[:, b])
        nc.vector.tensor_mul(probs[:, b], ef_ap, rs_ap)
    nc.scalar.activation(dumout, dumin, AF.Sqrt)          # re-prefetch sqrt

    # ---- xp[d, (b,o)] += xnb_tile^T @ probs_tile (contract i) ----
    psxp = pp.tile([P, BO], F32, tag="xp")
    for b in range(BL):
        for it in range(ITI):
            nc.tensor.matmul(psxp[:, b * O:(b + 1) * O],
                             xnb[:, b * ITI + it, :], probs[:, b, it, :],
                             start=(it == 0), stop=(it == ITI - 1))
    xps = data.tile([P, BO], F16)
    for q in range(4):
        nc.vector.tensor_copy(xps[:, q * 64:(q + 1) * 64],
                              psxp[:, q * 64:(q + 1) * 64])

    # ---- v1[h, (o,b)] = wfb_o^T @ xp_o (contract d), o-major columns so
    # the squash chain runs per o-half, overlapping the second half's mms ----
    psv1 = pp.tile([H, BO], F32, tag="v0")
    xpsv = xps.rearrange("d (b o) -> d o b", o=O)
    for o in range(O):
        nc.tensor.matmul(psv1[:, o * BL:(o + 1) * BL], wfb[:, o, :],
                         xpsv[:, o, :], start=True, stop=True)

    # squash(v1): sq1 = (v1/64)^2 on ACT; g1 = 64*sqrt(sn1')/(1+4096*sn1')
    # restores the scaling exactly (sn1' = sn1/4096): out1 = v1 * g1.
    sq1 = data.tile([H, BO], F16)
    psn1 = pp.tile([H, BO], F32, tag="g")
    rt1 = data.tile([H, BO], F32)
    dn1 = data.tile([H, BO], F32)
    rdn1 = data.tile([H, BO], F32)
    g1 = data.tile([H, BO], F32)
    out1 = data.tile([H, BO], F16)
    for hf in range(2):
        sl = slice(hf * P, (hf + 1) * P)
        nc.scalar.activation(sq1[:, sl], psv1[:, sl], AF.Square,
                             scale=1.0 / 64)
        nc.tensor.matmul(psn1[:, sl], ones64, sq1[:, sl],
                         start=True, stop=True)
        nc.scalar.activation(rt1[:, sl], psn1[:, sl], AF.Sqrt)
        nc.vector.tensor_scalar(dn1[:, sl], psn1[:, sl], 64.0, 1.0 / 64,
                                op0=mybir.AluOpType.mult,
                                op1=mybir.AluOpType.add)
        nc.vector.reciprocal(rdn1[:, sl], dn1[:, sl])
        nc.vector.tensor_mul(g1[:, sl], rt1[:, sl], rdn1[:, sl])
        nc.vector.tensor_mul(out1[:, sl], psv1[:, sl], g1[:, sl])

    # ---- transpose [h, (o,b)] -> [(o,b), h]; the DMA access pattern then
    # scatters the o-major rows into the [b, o, h] DRAM layout ----
    outT = data.tile([H, BO // H, H], F32)
    # ovv[o2, b, t, h] = out_d[b, t*8+o2, h]; flattened (o2,b,t,h) order
    # matches outT's (c=(o2,b), t, h) order element-for-element.
    ovv = out_d.rearrange("b (t o2) h -> o2 b t h", o2=BL)
    for t in range(BO // H):
        pso = pp.tile([H, H], F16, tag="w2", bufs=2)
        nc.tensor.transpose(pso, out1[:, t * H:(t + 1) * H], ident)
        nc.vector.tensor_copy(outT[:, t, :], pso)
    for t in range(BO // H):
        eng = nc.sync if t % 2 == 0 else nc.scalar
        eng.dma_start(out=ovv[:, :, t, :], in_=outT[:, t, :])


def build_program():
    nc = bacc.Bacc("TRN2", debug=False, num_devices=NCORES)
    xtb_t = nc.dram_tensor("xtb", [P, BL, I], F16, kind="ExternalInput")
    xnb_t = nc.dram_tensor("xnb", [P, BL * ITI, DIN], F16,
                           kind="ExternalInput")
    wfb_t = nc.dram_tensor("wfb", [P, O, H], F16, kind="ExternalInput")
    wtbP_t = nc.dram_tensor("wtbP", [P, NJ, DIN], F16, kind="ExternalInput")
    out_t = nc.dram_tensor("out", [BL, O, H], F32, kind="ExternalOutput")
    with tile.TileContext(nc) as tc:
        capsule_tile_kernel(tc, out_t.ap(), xtb_t.ap(), xnb_t.ap(),
                            wfb_t.ap(), wtbP_t.ap())
    nc.compile()
    return nc


_program = None


def _get_program():
    global _program
    if _program is None:
        _program = build_program()
    return _program


def run_on_cores(x, route_weights, trace=False, **kwargs):
    """Run the SPMD kernel; returns (full_output, BassKernelResults)."""
    x = np.asarray(x, dtype=np.float32).astype(NPF16)
    w = np.asarray(route_weights, dtype=np.float32).astype(NPF16)
    nc = _get_program()
    wfb = np.ascontiguousarray(w.transpose(1, 0, 2))
    wtbP = np.ascontiguousarray(
        w.reshape(NJ, 2, DIN, H).transpose(1, 3, 0, 2).reshape(P, NJ, DIN))
    in_maps = []
    for c in range(NCORES):
        xs = x[c * BL:(c + 1) * BL]
        xtb = np.ascontiguousarray(xs.transpose(2, 0, 1))
        xnb = np.ascontiguousarray(
            xs.reshape(BL * ITI, P, DIN).transpose(1, 0, 2))
        in_maps.append({"xtb": xtb, "xnb": xnb, "wfb": wfb, "wtbP": wtbP})
    res = bass_utils.run_bass_kernel_spmd(
        nc, in_maps, core_ids=list(range(NCORES)), trace=trace, **kwargs
    )
    out = np.concatenate([res.results[c]["out"] for c in range(NCORES)], axis=0)
    return out.astype(np.float32), res


def kernel(x, route_weights):
    out, _ = run_on_cores(x, route_weights)
    return out
